# revision 1
# baseline (speedup 1.0000x reference)
"""Trainium2 Bass kernel for DeformConv2D (b=4, c=64, H=W=128, ks=3).

Sharding: 8 cores = (sample s = core//2) x (row-half = core%2). Each core
computes output rows [64*half, 64*half+64) of its sample.

Per-core dataflow:
  1. Load a 74-row halo slice of x (f32, CHW) into SBUF, zero-padded cols.
  2. Build XD in DRAM: bf16 [74*132 slots, 2 cols, 64 ch] -- HWC layout with
     duplicated column pairs so one 256B gather element = (2 cols x 64 ch).
  3. Offset conv on PE (9 taps, K=64 matmuls) -> offsets [18, 8192] in PSUM.
  4. PE-transpose offsets to [128 w, 64 t, 18]; DVE coordinate pipeline
     computes bilinear corner weights W4 and int16 gather indices (wrapped
     16-partition layout for dma_gather, staged via DRAM).
  5. dma_gather (36 calls: 9 kernel points x 4 t-chunks) from XD.
  6. DVE combine: xoff[w, t, n, ci] = sum_rc W4 * G   (TS + 3 STT per (t,n)).
  7. PE-transpose xoff -> [(n, ci), pix]; final conv = 5 accumulating
     matmuls (K=576 over (n, ci)) -> out [64 co, 128 pix] per row.
"""
import sys
import types
import numpy as np
import ml_dtypes

sys.path.insert(0, "/opt/trn_rl_repo")

BF16 = ml_dtypes.bfloat16
NCORES = 8
NR = 74          # XD rows (local): row k <-> abs padded row h0-4+k
WROW = 132       # XD row stride in 256B elements
NSLOT = NR * WROW


def _install_ntff_hook():
    if "antenv.axon_hooks" in sys.modules:
        return
    try:
        import antenv
        from trn_agent_boot.trn_boot import _ntff_profile_via_ctypes
    except Exception:
        return
    mod = types.ModuleType("antenv.axon_hooks")
    _hook = [None]
    mod.set_axon_ntff_profile_hook = lambda h: _hook.__setitem__(0, h)
    mod.get_axon_ntff_profile_hook = lambda: _hook[0]
    sys.modules["antenv.axon_hooks"] = mod
    antenv.axon_hooks = mod
    try:
        mod.set_axon_ntff_profile_hook(
            _ntff_profile_via_ctypes("/opt/axon/libaxon_pjrt.so"))
    except Exception:
        mod.set_axon_ntff_profile_hook(None)


_PROGRAM = None


def _build_program():
    global _PROGRAM
    if _PROGRAM is not None:
        return _PROGRAM
    from contextlib import ExitStack
    import concourse.bass as bass
    import concourse.tile as tile
    from concourse import mybir, bacc

    f32 = mybir.dt.float32
    bf16 = mybir.dt.bfloat16
    i16 = mybir.dt.int16
    i32 = mybir.dt.int32
    A = mybir.AluOpType

    nc = bacc.Bacc()
    # ---- I/O ----
    xg_p = nc.declare_dram_parameter("xg", [64, NR * 128], f32, isOutput=False)
    base2_p = nc.declare_dram_parameter("base2", [128, 64 * 18], f32, isOutput=False)
    xsc_p = nc.declare_dram_parameter("xsc", [128, 4], f32, isOutput=False)
    woff_p = nc.declare_dram_parameter("woff", [64, 9 * 18], f32, isOutput=False)
    wca_p = nc.declare_dram_parameter("wconv_a", [128, 256], bf16, isOutput=False)
    wcb_p = nc.declare_dram_parameter("wconv_b", [64, 64], bf16, isOutput=False)
    idf_p = nc.declare_dram_parameter("ident_f", [128, 128], f32, isOutput=False)
    idb_p = nc.declare_dram_parameter("ident_b", [128, 128], bf16, isOutput=False)
    out_p = nc.declare_dram_parameter("out", [64, 64 * 128], f32, isOutput=True)

    xd = nc.dram_tensor("xd", [NSLOT, 128], bf16)           # gather source
    gstage = nc.dram_tensor("gstage", [16, 9216], i16)      # idx staging

    with tile.TileContext(nc) as tc, ExitStack() as ctx:
        consts = ctx.enter_context(tc.tile_pool(name="consts", bufs=1))
        big = ctx.enter_context(tc.tile_pool(name="big", bufs=1))
        scratch = ctx.enter_context(tc.tile_pool(name="scratch", bufs=4))
        ps_seq = ctx.enter_context(tc.tile_pool(name="ps_seq", bufs=3, space="PSUM"))
        co_ctx = ExitStack()
        coords = co_ctx.enter_context(tc.tile_pool(name="coords", bufs=1))

        # ---------- load constants ----------
        base2 = consts.tile([128, 64 * 18], f32)
        nc.sync.dma_start(out=base2, in_=base2_p[:, :])
        xsc = consts.tile([128, 4], f32)
        nc.sync.dma_start(out=xsc, in_=xsc_p[:, :])
        woff = consts.tile([64, 9, 18], f32)
        nc.sync.dma_start(out=woff, in_=woff_p[:, :].rearrange("a (t c) -> a t c", t=9))
        wca = consts.tile([128, 256], bf16)
        nc.sync.dma_start(out=wca, in_=wca_p[:, :])
        wcb = consts.tile([64, 64], bf16)
        nc.sync.dma_start(out=wcb, in_=wcb_p[:, :])
        idf = consts.tile([128, 128], f32)
        nc.sync.dma_start(out=idf, in_=idf_p[:, :])
        idb = consts.tile([128, 128], bf16)
        nc.sync.dma_start(out=idb, in_=idb_p[:, :])

        # ---------- phase A: x load + XD build ----------
        ab_ctx = ExitStack()
        abp = ab_ctx.enter_context(tc.tile_pool(name="abp", bufs=1))
        xsb = abp.tile([64, NR, WROW], f32, name="xsb")  # padded CHW slab
        nc.vector.memset(xsb, 0.0)
        nc.sync.dma_start(
            out=xsb[:, :, 1:129],
            in_=xg_p[:, :].rearrange("c (r w) -> c r w", r=NR))

        # zero-fill whole XD (borders + unwritten rows)
        zb = abp.tile([128, NSLOT // 8], bf16, name='zb')       # 9768/8 = 1221 per part
        nc.vector.memset(zb, 0.0)
        for c8 in range(8):
            nc.sync.dma_start(
                out=bass.AP(tensor=xd, offset=c8 * (NSLOT // 8),
                            ap=[[NSLOT, 128], [1, NSLOT // 8]]),
                in_=zb)

        # transpose x rows -> bf16 HWC, write slot0/slot1 into XD
        for blk in range(10):                        # 8 rows per block; 74 rows
            rows = min(8, NR - blk * 8)
            pst = ps_seq.tile([128, 512], f32, tag="seq")
            for j in range(rows):
                k = blk * 8 + j
                nc.tensor.transpose(
                    pst[:, j * 64:(j + 1) * 64], xsb[:, k, 1:129],
                    idf[0:64, 0:64])
            xrb = scratch.tile([128, 8, 64], bf16, tag="xrow")
            nc.any.tensor_copy(
                xrb[:, 0:rows, :],
                pst[:, 0:rows * 64].rearrange("p (r c) -> p r c", r=rows))
            # slot0: XD[k, w, 0, :] = x[., k, w] for w=1..128 (padded col idx)
            nc.sync.dma_start(
                out=bass.AP(tensor=xd,
                            offset=(blk * 8 * WROW + 1) * 128 + 0,
                            ap=[[128, 128], [WROW * 128, rows], [1, 64]]),
                in_=xrb[:, 0:rows, :])
            # slot1: XD[k, w-1, 1, :] = x[., k, w]
            nc.sync.dma_start(
                out=bass.AP(tensor=xd,
                            offset=(blk * 8 * WROW + 0) * 128 + 64,
                            ap=[[128, 128], [WROW * 128, rows], [1, 64]]),
                in_=xrb[:, 0:rows, :])

        # ---------- phase B: offset conv ----------
        off_sb = abp.tile([18, 64, 128], f32, name='off_sb')
        for tb in range(16):                         # 4 output rows per tile
            psc = ps_seq.tile([18, 512], f32, tag="seq")
            for dy in range(3):
                for dx in range(3):
                    tap = dy * 3 + dx
                    nc.tensor.matmul(
                        psc[:, :],
                        woff[:, tap, :],
                        bass.AP(tensor=xsb.tensor,
                                offset=xsb.offset + (tb * 4 + dy + 4) * WROW + dx,
                                ap=[xsb.ap[0], [WROW, 4], [1, 128]]),
                        start=(tap == 0), stop=(tap == 8))
            nc.any.tensor_copy(
                off_sb[:, tb * 4:tb * 4 + 4, :],
                psc[:, :].rearrange("p (r w) -> p r w", r=4))

        # transpose offsets -> offt [128 w, 64 t, 18]
        offt = coords.tile([128, 64, 18], f32)
        for b in range(4):
            pst = ps_seq.tile([128, 288], f32, tag="seq")
            for j in range(16):
                t = b * 16 + j
                nc.tensor.transpose(
                    pst[:, j * 18:(j + 1) * 18],
                    off_sb[:, t, :], idf[0:18, 0:18])
            nc.any.tensor_copy(
                offt[:, b * 16:(b + 1) * 16, :],
                pst[:, :].rearrange("p (t c) -> p t c", t=16))
        ab_ctx.close()

        # ---------- phase C: coordinates ----------
        def cT(shape, tag):
            return coords.tile(shape, f32, tag=tag, name=tag)

        P = cT([128, 64, 18], "P")
        nc.vector.tensor_tensor(
            P, offt, base2.rearrange("p (t c) -> p t c", t=64), A.add)
        q_i = coords.tile([128, 64, 18], i32, tag="cs", name="qi", bufs=4)
        nc.vector.tensor_copy(q_i, P)
        Qf0 = coords.tile([128, 64, 18], f32, tag="cs", name="qf0", bufs=4)
        nc.vector.tensor_copy(Qf0, q_i)
        GT = coords.tile([128, 64, 18], f32, tag="cs", name="gt", bufs=4)
        nc.vector.tensor_tensor(GT, Qf0, P, A.is_gt)
        Qf = cT([128, 64, 18], "qf")
        nc.vector.tensor_tensor(Qf, Qf0, GT, A.subtract)
        FR = coords.tile([128, 64, 18], f32, tag="cs", name="fr", bufs=4)
        nc.vector.tensor_tensor(FR, P, Qf, A.subtract)
        INR = coords.tile([128, 64, 18], f32, tag="cs", name="inr", bufs=4)
        # x half: per-core bounds via scalar APs; y half: immediates
        nc.vector.tensor_scalar(INR[:, :, 0:9], P[:, :, 0:9],
                                xsc[:, 0:1], None, A.is_ge)
        nc.vector.tensor_scalar(INR[:, :, 9:18], P[:, :, 9:18],
                                9.0, None, A.is_ge)
        INH = coords.tile([128, 64, 18], f32, tag="cs", name="inh", bufs=4)
        nc.vector.tensor_scalar(INH[:, :, 0:9], P[:, :, 0:9],
                                xsc[:, 1:2], None, A.is_le)
        nc.vector.tensor_scalar(INH[:, :, 9:18], P[:, :, 9:18],
                                136.0, None, A.is_le)
        nc.vector.tensor_tensor(INR, INR, INH, A.mult)
        FRV = cT([128, 64, 18], "frv")
        nc.vector.tensor_tensor(FRV, FR, INR, A.mult)
        ALT = cT([128, 64, 18], "alt")
        nc.vector.tensor_scalar(ALT, FRV, -1.0, 1.0, A.mult, A.add)
        QC = cT([128, 64, 18], "qc")
        nc.vector.tensor_scalar(QC[:, :, 0:9], Qf[:, :, 0:9],
                                xsc[:, 2:3], xsc[:, 3:4], A.max, A.min)
        nc.vector.tensor_scalar(QC[:, :, 9:18], Qf[:, :, 9:18],
                                8.0, 137.0, A.max, A.min)
        # gather linear indices (f32 exact ints)
        LINF = cT([128, 64, 9], "linf")
        nc.vector.tensor_scalar(LINF, QC[:, :, 0:9], 132.0, -536.0, A.mult, A.add)
        nc.vector.tensor_tensor(LINF, LINF, QC[:, :, 9:18], A.add)
        LIN2 = coords.tile([128, 9, 64, 2], f32, tag="lin2", name="lin2")
        linf_T = bass.AP(tensor=LINF.tensor, offset=LINF.offset,
                         ap=[LINF.ap[0], [1, 9], [9, 64]])
        nc.vector.tensor_copy(LIN2[:, :, :, 0], linf_T)
        nc.vector.tensor_scalar(LIN2[:, :, :, 1], linf_T, 132.0, None, A.add)
        gidx_pre = coords.tile([128, 9, 64, 2], i16, tag="gpre", name="gpre")
        nc.vector.tensor_copy(gidx_pre, LIN2)
        # corner weight products [128, 64t, 9n, 4rc]
        W4 = consts.tile([128, 64, 9, 4], f32, tag="w4", name="w4")
        nc.vector.tensor_tensor(W4[:, :, :, 0], ALT[:, :, 0:9], ALT[:, :, 9:18], A.mult)
        nc.vector.tensor_tensor(W4[:, :, :, 1], ALT[:, :, 0:9], FRV[:, :, 9:18], A.mult)
        nc.vector.tensor_tensor(W4[:, :, :, 2], FRV[:, :, 0:9], ALT[:, :, 9:18], A.mult)
        nc.vector.tensor_tensor(W4[:, :, :, 3], FRV[:, :, 0:9], FRV[:, :, 9:18], A.mult)

        # ---------- idx relayout to wrapped-16 (via DRAM staging) ----------
        # gstage[pl, n, tc, tt, r, ph] = gidx_pre[ph*16+pl, tc*16+tt, n, r]
        for ph in range(8):
            sl = gidx_pre[ph * 16:ph * 16 + 16]
            nc.sync.dma_start(
                out=bass.AP(tensor=gstage, offset=ph * 1152,
                            ap=[[9216, 16], [1, 1152]]),
                in_=bass.AP(tensor=sl.tensor, offset=sl.offset,
                            ap=[sl.ap[0], [1, 1152]]))
        sg = consts.tile([128, 8, 1152], i16, name="sg")
        nc.gpsimd.dma_start(
            out=sg,
            in_=bass.AP(tensor=gstage, offset=0,
                        ap=[[0, 8], [9216, 16], [1, 9216]]))
        gidx = consts.tile([128, 9, 4, 256], i16)
        # ph-interleave on DVE: gidx[p, j2*8+ph] = sg[p, ph, j2]
        nc.vector.tensor_copy(
            bass.AP(tensor=gidx.tensor, offset=gidx.offset,
                    ap=[gidx.ap[0], [1, 8], [8, 1152]]),
            sg)

        # pre-drain gather deps onto the Pool engine (the DMA-gather ISA
        # struct supports very few semaphore waits)
        j1 = scratch.tile([16, 8], bf16, tag="join", name="j1")
        nc.sync.dma_start(out=j1[0:1, 0:8], in_=xd[0:1, 0:8])
        j2 = scratch.tile([16, 8], i16, tag="join2", name="j2")
        j3 = scratch.tile([16, 8], bf16, tag="join3", name="j3")
        nc.gpsimd.tensor_copy(j2[0:16, 0:4], gidx[0:16, 0, 0, 0:4])
        nc.gpsimd.tensor_copy(j3[0:1, 0:4], j1[0:1, 0:4])

        # ---------- phase D: gather + combine + final conv ----------
        co_ctx.close()
        ps_x = ctx.enter_context(tc.tile_pool(name="ps_x", bufs=2, space="PSUM"))
        ps_o = ctx.enter_context(tc.tile_pool(name="ps_o", bufs=2, space="PSUM"))
        gpool = ctx.enter_context(tc.tile_pool(name="gpool", bufs=3))
        xpool = ctx.enter_context(tc.tile_pool(name="xpool", bufs=2))
        tpool = ctx.enter_context(tc.tile_pool(name="tpool", bufs=8))
        rpool = ctx.enter_context(tc.tile_pool(name="rpool", bufs=3))
        for tcn in range(4):                         # t-chunks of 16 rows
            outb = big.tile([64, 16, 128], f32, tag="outb", bufs=2, name="outb")
            xoff = xpool.tile([128, 16, 9, 64], bf16, tag="xoff", name="xoff")
            gs = []
            for n in range(9):
                g = gpool.tile([128, 16, 2, 2, 64], bf16, tag="g")
                nc.gpsimd.dma_gather(
                    out_ap=g.rearrange("p a b c d -> p (a b) (c d)"),
                    in_ap=xd[:, :],
                    idxs_ap=gidx[:, n, tcn, :],
                    num_idxs=4096,
                    num_idxs_reg=4096,
                    elem_size=128,
                    single_packet=False,
                )
                gs.append(g)
            for n in range(9):
                g = gs[n]
                for tt in range(16):
                    t = tcn * 16 + tt
                    tmp = tpool.tile([128, 64], bf16, tag="tmp")
                    nc.vector.tensor_scalar(
                        tmp, g[:, tt, 0, 0, :], W4[:, t, n, 0:1], None, A.mult)
                    tmp2 = tpool.tile([128, 64], bf16, tag="tmp")
                    nc.vector.scalar_tensor_tensor(
                        tmp2, g[:, tt, 0, 1, :], W4[:, t, n, 1:2], tmp,
                        A.mult, A.add)
                    tmp3 = tpool.tile([128, 64], bf16, tag="tmp")
                    nc.vector.scalar_tensor_tensor(
                        tmp3, g[:, tt, 1, 0, :], W4[:, t, n, 2:3], tmp2,
                        A.mult, A.add)
                    nc.vector.scalar_tensor_tensor(
                        xoff[:, tt, n, :], g[:, tt, 1, 1, :], W4[:, t, n, 3:4],
                        tmp3, A.mult, A.add)
            # transpose xoff per row, final conv
            for tt in range(16):
                t = tcn * 16 + tt
                pso = ps_o.tile([64, 128], f32, tag="o")
                for jc in range(4):
                    psx = ps_x.tile([128, 128], bf16, tag="x")
                    nc.tensor.transpose(
                        psx,
                        xoff[:, tt, 2 * jc:2 * jc + 2, :].rearrange(
                            "p a b -> p (a b)"),
                        idb)
                    rhs = rpool.tile([128, 128], bf16, tag="r")
                    nc.any.tensor_copy(rhs, psx)
                    nc.tensor.matmul(pso, wca[:, jc * 64:(jc + 1) * 64], rhs,
                     start=(jc == 0), stop=False)
                psx4 = ps_x.tile([128, 128], bf16, tag="x")
                nc.tensor.transpose(
                    psx4[0:64, :], xoff[:, tt, 8, :], idb)
                rhs4 = rpool.tile([64, 128], bf16, tag="r4")
                nc.any.tensor_copy(rhs4, psx4[0:64, :])
                nc.tensor.matmul(pso, wcb, rhs4, start=False, stop=True)
                nc.any.tensor_copy(outb[:, tt, :], pso)

            nc.sync.dma_start(
                out=out_p[:, tcn * 2048:(tcn + 1) * 2048],
                in_=outb.rearrange("c t w -> c (t w)"))

    nc.finalize()
    _PROGRAM = nc
    return nc


def _host_consts(W_off, b_off, W_conv):
    idxr = np.concatenate([np.arange(0, 18, 2), np.arange(1, 18, 2)])
    W_off_r = W_off[idxr]            # (18, 64, 3, 3)
    b_off_r = b_off[idxr]            # (18,)
    woff = np.ascontiguousarray(
        W_off_r.transpose(2, 3, 1, 0).reshape(9, 64, 18).transpose(1, 0, 2)
    ).reshape(64, 9 * 18).astype(np.float32)
    # base2 [128 w, 64 t, 18]
    nidx = np.arange(9)
    pnx = (nidx // 3) - 1
    pny = (nidx % 3) - 1
    tt = np.arange(64)
    ww = np.arange(128)
    base2 = np.zeros((128, 64, 18), np.float32)
    base2[:, :, 0:9] = tt[None, :, None] + 9 + pnx[None, None, :] + \
        b_off_r[None, None, 0:9]
    base2[:, :, 9:18] = ww[:, None, None] + 9 + pny[None, None, :] + \
        b_off_r[None, None, 9:18]
    base2 = base2.reshape(128, 64 * 18)
    # final conv weights
    Wmat = W_conv.reshape(64, 64, 9).transpose(0, 2, 1)   # (co, n, ci)
    wca = np.zeros((128, 256), np.float32)
    for jc in range(4):
        for dn in range(2):
            # K row = dn*64+ci ; col block jc : [K, co]
            wca[dn * 64:(dn + 1) * 64, jc * 64:(jc + 1) * 64] = \
                Wmat[:, 2 * jc + dn, :].T
    wcb = np.ascontiguousarray(Wmat[:, 8, :].T)           # (ci, co)
    return {
        "woff": woff,
        "base2": base2,
        "wconv_a": wca.astype(BF16),
        "wconv_b": wcb.astype(BF16),
        "ident_f": np.eye(128, dtype=np.float32),
        "ident_b": np.eye(128, dtype=np.float32).astype(BF16),
    }


def _per_core_inputs(x, consts, s, half):
    h0 = 64 * half
    xs = x[s]                                    # (64, 128, 128)
    xgs = np.zeros((64, NR, 128), np.float32)
    lo = h0 - 5                                  # unpadded row of xg row 0
    for k in range(NR):
        r = lo + k
        if 0 <= r < 128:
            xgs[:, k, :] = xs[:, r, :]
    xsc = np.zeros((128, 4), np.float32)
    xsc[:, 0] = 9 - h0                           # mask lo
    xsc[:, 1] = 136 - h0                         # mask hi
    xsc[:, 2] = 8 - min(h0, 2)                   # clip lo (tightened)
    xsc[:, 3] = min(129, h0 + 69) - h0 + 8       # clip hi (tightened)
    return {
        "xg": xgs.reshape(64, NR * 128),
        "xsc": xsc,
        **consts,
    }


def kernel(x, W_off, b_off, W_conv):
    _install_ntff_hook()
    # the bass kernel must run on the axon trn2 backend; undo any cpu pin
    # (e.g. a harness that set JAX_PLATFORMS=cpu for the reference)
    import os
    if os.environ.get("JAX_PLATFORMS", "") == "cpu":
        try:
            import jax
            jax.config.update("jax_platforms", None)
            os.environ.pop("JAX_PLATFORMS", None)
        except Exception:
            pass
    x = np.asarray(x, np.float32)
    W_off = np.asarray(W_off, np.float32)
    b_off = np.asarray(b_off, np.float32)
    W_conv = np.asarray(W_conv, np.float32)

    from concourse.bass_utils import run_bass_kernel_spmd
    nc = _build_program()
    consts = _host_consts(W_off, b_off, W_conv)
    in_maps = [
        _per_core_inputs(x, consts, core // 2, core % 2) for core in range(NCORES)
    ]
    res = run_bass_kernel_spmd(nc, in_maps, list(range(NCORES)))
    out = np.empty((4, 64, 128, 128), np.float32)
    for core in range(NCORES):
        s, half = core // 2, core % 2
        out[s, :, 64 * half:64 * half + 64, :] = \
            res.results[core]["out"].reshape(64, 64, 128)
    return out



# revision 4
# speedup vs baseline: 2.0166x; 2.0166x over previous
"""Trainium2 Bass kernel for DeformConv2D (b=4, c=64, H=W=128, ks=3).

Sharding: 8 cores = (sample s = core//2) x (row-half = core%2). Each core
computes output rows [64*half, 64*half+64) of its sample.

v2 dataflow (per core), pipelined over 4 t-chunks of 16 rows:
  A. Load a 74-row bf16 halo slab of x (CHW) into SBUF; build XD2 in DRAM:
     [74*132 slots, 512B] where slot (k,c) = 2rows x 2cols x 64ch -- so ONE
     gather descriptor fetches a full bilinear corner block per sample.
  B. Offset conv on PE (bf16, 9 taps, K=64) -> offsets [18, 16, 128] f32;
     PE-transpose to [128w, 16t, 18].
  C. DVE coordinate pipeline: floor/frac/masks, single linear slot index
     per sample; int16 idx relayout to the gather's wrapped-16 layout via
     a DRAM staging roundtrip.
  D. 3 dma_gathers (3 kernel points each, 6144 idxs, 512B elems); DVE
     combine = lerp-of-lerp (batched corner deltas + 2 fused madds per
     row); PE transpose + final conv as 512-wide accumulating matmuls.
"""
import sys
import types
import numpy as np
import ml_dtypes

sys.path.insert(0, "/opt/trn_rl_repo")

BF16 = ml_dtypes.bfloat16
NCORES = 8
NR = 74          # slab rows (local): row k <-> unpadded row h0-5+k
WC = 132         # slab/XD2 col count
NSLOT = NR * WC  # 9768


def _install_ntff_hook():
    if "antenv.axon_hooks" in sys.modules:
        return
    try:
        import antenv
        from trn_agent_boot.trn_boot import _ntff_profile_via_ctypes
    except Exception:
        return
    mod = types.ModuleType("antenv.axon_hooks")
    _hook = [None]
    mod.set_axon_ntff_profile_hook = lambda h: _hook.__setitem__(0, h)
    mod.get_axon_ntff_profile_hook = lambda: _hook[0]
    sys.modules["antenv.axon_hooks"] = mod
    antenv.axon_hooks = mod
    try:
        mod.set_axon_ntff_profile_hook(
            _ntff_profile_via_ctypes("/opt/axon/libaxon_pjrt.so"))
    except Exception:
        mod.set_axon_ntff_profile_hook(None)


_PROGRAM = None


def _build_program():
    global _PROGRAM
    if _PROGRAM is not None:
        return _PROGRAM
    from contextlib import ExitStack
    import concourse.bass as bass
    import concourse.tile as tile
    from concourse import mybir, bacc

    f32 = mybir.dt.float32
    bf16 = mybir.dt.bfloat16
    i16 = mybir.dt.int16
    i32 = mybir.dt.int32
    A = mybir.AluOpType

    nc = bacc.Bacc(num_swdge_queues=4)
    # ---- I/O ----
    xg_p = nc.declare_dram_parameter("xg", [64, NR * 128], bf16, isOutput=False)
    base2_p = nc.declare_dram_parameter("base2", [128, 64 * 18], f32, isOutput=False)
    xsc_p = nc.declare_dram_parameter("xsc", [128, 2], f32, isOutput=False)
    woff_p = nc.declare_dram_parameter("woff", [64, 9 * 18], bf16, isOutput=False)
    wca_p = nc.declare_dram_parameter("wconv_a", [128, 256], bf16, isOutput=False)
    wcb_p = nc.declare_dram_parameter("wconv_b", [64, 64], bf16, isOutput=False)
    idf_p = nc.declare_dram_parameter("ident_f", [128, 128], f32, isOutput=False)
    idb_p = nc.declare_dram_parameter("ident_b", [128, 128], bf16, isOutput=False)
    out_p = nc.declare_dram_parameter("out", [64, 64 * 128], f32, isOutput=True)

    xd2 = nc.dram_tensor("xd2", [NSLOT, 256], bf16)        # gather source
    gstage = nc.dram_tensor("gstage", [16, 4 * 1152], i16)  # idx staging

    with tile.TileContext(nc) as tc, ExitStack() as ctx:
        consts = ctx.enter_context(tc.tile_pool(name="consts", bufs=1))
        slab = ctx.enter_context(tc.tile_pool(name="slab", bufs=1))

        # ---------- load constants ----------
        base2 = consts.tile([128, 64, 18], f32)
        nc.sync.dma_start(out=base2,
                          in_=base2_p[:, :].rearrange("a (t c) -> a t c", t=64))
        xsc = consts.tile([128, 2], f32)
        nc.sync.dma_start(out=xsc, in_=xsc_p[:, :])
        woff = consts.tile([64, 9, 18], bf16)
        nc.sync.dma_start(out=woff, in_=woff_p[:, :].rearrange("a (t c) -> a t c", t=9))
        wca = consts.tile([128, 256], bf16)
        nc.sync.dma_start(out=wca, in_=wca_p[:, :])
        wcb = consts.tile([64, 64], bf16)
        nc.sync.dma_start(out=wcb, in_=wcb_p[:, :])
        idf = consts.tile([128, 128], f32)
        nc.sync.dma_start(out=idf, in_=idf_p[:, :])
        idb = consts.tile([128, 128], bf16)
        nc.sync.dma_start(out=idb, in_=idb_p[:, :])

        # ---------- phase A: x slab load + XD2 build ----------
        xsb = slab.tile([64, NR, WC], bf16, name="xsb")
        nc.vector.memset(xsb, 0.0)
        nc.sync.dma_start(
            out=xsb[:, :, 1:129],
            in_=xg_p[:, :].rearrange("c (r w) -> c r w", r=NR))

        a_ctx = ExitStack()
        apool = a_ctx.enter_context(tc.tile_pool(name="apool", bufs=1))
        aps = a_ctx.enter_context(tc.tile_pool(name="aps", bufs=2, space="PSUM"))
        ascr = a_ctx.enter_context(tc.tile_pool(name="ascr", bufs=3))

        zb = apool.tile([128, 2442], bf16, name="zb")
        nc.vector.memset(zb, 0.0)
        for c8 in range(8):
            nc.sync.dma_start(
                out=bass.AP(tensor=xd2, offset=c8 * 312576,
                            ap=[[2442, 128], [1, 2442]]),
                in_=zb)

        # transpose x rows -> bf16 HWC, write the 4 sub-blocks of XD2
        for blk in range(10):
            rows = min(8, NR - blk * 8)
            k0 = blk * 8
            pst = aps.tile([128, 8, 64], bf16, tag="ta")
            for j in range(rows):
                nc.tensor.transpose(
                    pst[:, j, :], xsb[:, k0 + j, 1:129], idb[0:64, 0:64])
            xrb = ascr.tile([128, 8, 64], bf16, tag="xrow")
            nc.any.tensor_copy(xrb[:, 0:rows, :], pst[:, 0:rows, :])
            # sub (0,0): slot (k, c=w+1) bytes [0,64)
            nc.sync.dma_start(
                out=bass.AP(tensor=xd2, offset=(k0 * WC + 1) * 256 + 0,
                            ap=[[256, 128], [WC * 256, rows], [1, 64]]),
                in_=xrb[:, 0:rows, :])
            # sub (0,1): slot (k, c=w) bytes [64,128)
            nc.sync.dma_start(
                out=bass.AP(tensor=xd2, offset=(k0 * WC + 0) * 256 + 64,
                            ap=[[256, 128], [WC * 256, rows], [1, 64]]),
                in_=xrb[:, 0:rows, :])
            # sub (1,0): slot (k-1, c=w+1) bytes [128,192)
            if blk == 0:
                nc.sync.dma_start(
                    out=bass.AP(tensor=xd2, offset=(0 * WC + 1) * 256 + 128,
                                ap=[[256, 128], [WC * 256, rows - 1], [1, 64]]),
                    in_=xrb[:, 1:rows, :])
                nc.sync.dma_start(
                    out=bass.AP(tensor=xd2, offset=(0 * WC + 0) * 256 + 192,
                                ap=[[256, 128], [WC * 256, rows - 1], [1, 64]]),
                    in_=xrb[:, 1:rows, :])
            else:
                nc.sync.dma_start(
                    out=bass.AP(tensor=xd2, offset=((k0 - 1) * WC + 1) * 256 + 128,
                                ap=[[256, 128], [WC * 256, rows], [1, 64]]),
                    in_=xrb[:, 0:rows, :])
                nc.sync.dma_start(
                    out=bass.AP(tensor=xd2, offset=((k0 - 1) * WC + 0) * 256 + 192,
                                ap=[[256, 128], [WC * 256, rows], [1, 64]]),
                    in_=xrb[:, 0:rows, :])
        a_ctx.close()

        # ---------- pools for phases B/C/D ----------
        ps_c = ctx.enter_context(tc.tile_pool(name="ps_c", bufs=2, space="PSUM"))
        ps_t = ctx.enter_context(tc.tile_pool(name="ps_t", bufs=2, space="PSUM"))
        ps_x = ctx.enter_context(tc.tile_pool(name="ps_x", bufs=2, space="PSUM"))
        ps_o = ctx.enter_context(tc.tile_pool(name="ps_o", bufs=2, space="PSUM"))
        bpool = ctx.enter_context(tc.tile_pool(name="bpool", bufs=2))
        cpool = ctx.enter_context(tc.tile_pool(name="cpool", bufs=2))
        tmpp = ctx.enter_context(tc.tile_pool(name="tmpp", bufs=4))
        ipool = ctx.enter_context(tc.tile_pool(name="ipool", bufs=2))
        gpool = ctx.enter_context(tc.tile_pool(name="gpool", bufs=2))
        dpool = ctx.enter_context(tc.tile_pool(name="dpool", bufs=2))
        xopool = ctx.enter_context(tc.tile_pool(name="xopool", bufs=2))
        rpool = ctx.enter_context(tc.tile_pool(name="rpool", bufs=2))
        opool = ctx.enter_context(tc.tile_pool(name="opool", bufs=2))
        jpool = ctx.enter_context(tc.tile_pool(name="jpool", bufs=4))

        # pre-drain XD2-ready onto Pool (gather ISA supports few sem waits)
        j1 = jpool.tile([16, 8], bf16, tag="j1", name="j1")
        nc.sync.dma_start(out=j1[0:1, 0:8], in_=xd2[0:1, 0:8])
        j3 = jpool.tile([16, 8], bf16, tag="j3", name="j3")
        nc.gpsimd.tensor_copy(j3[0:1, 0:4], j1[0:1, 0:4])

        for tcn in range(4):
            # ---------- phase B: offset conv (bf16) ----------
            off_sb = bpool.tile([18, 16, 128], f32, tag="offsb")
            for tb in range(4):
                psc = ps_c.tile([18, 512], f32, tag="c")
                for dy in range(3):
                    for dx in range(3):
                        tap = dy * 3 + dx
                        nc.tensor.matmul(
                            psc[:, :],
                            woff[:, tap, :],
                            bass.AP(tensor=xsb.tensor,
                                    offset=xsb.offset
                                    + (tcn * 16 + tb * 4 + dy + 4) * WC + dx,
                                    ap=[xsb.ap[0], [WC, 4], [1, 128]]),
                            start=(tap == 0), stop=(tap == 8))
                nc.any.tensor_copy(
                    off_sb[:, tb * 4:tb * 4 + 4, :],
                    psc[:, :].rearrange("p (r w) -> p r w", r=4))
            # transpose offsets -> offt [128 w, 16 t, 18]
            pst = ps_t.tile([128, 288], f32, tag="t")
            for j in range(16):
                nc.tensor.transpose(
                    pst[:, j * 18:(j + 1) * 18], off_sb[:, j, :],
                    idf[0:18, 0:18])
            offt = cpool.tile([128, 16, 18], f32, tag="offt")
            nc.any.tensor_copy(
                offt, pst[:, :].rearrange("p (t c) -> p t c", t=16))

            # ---------- phase C: coordinates ----------
            P = cpool.tile([128, 16, 18], f32, tag="P")
            nc.vector.tensor_tensor(P, offt, base2[:, tcn * 16:(tcn + 1) * 16, :],
                                    A.add)
            q_i = tmpp.tile([128, 16, 18], i32, tag="ct", name="qi")
            nc.vector.tensor_copy(q_i, P)
            Qf0 = tmpp.tile([128, 16, 18], f32, tag="ct", name="qf0")
            nc.vector.tensor_copy(Qf0, q_i)
            GT = tmpp.tile([128, 16, 18], f32, tag="ct", name="gt")
            nc.vector.tensor_tensor(GT, Qf0, P, A.is_gt)
            Qf = cpool.tile([128, 16, 18], f32, tag="Qf")
            nc.vector.tensor_tensor(Qf, Qf0, GT, A.subtract)
            FR = tmpp.tile([128, 16, 18], f32, tag="ct", name="fr")
            nc.vector.tensor_tensor(FR, P, Qf, A.subtract)
            INR = tmpp.tile([128, 16, 18], f32, tag="ct", name="inr")
            nc.vector.tensor_scalar(INR[:, :, 0:9], P[:, :, 0:9],
                                    xsc[:, 0:1], None, A.is_ge)
            nc.vector.tensor_scalar(INR[:, :, 9:18], P[:, :, 9:18],
                                    9.0, None, A.is_ge)
            INH = tmpp.tile([128, 16, 18], f32, tag="ct", name="inh")
            nc.vector.tensor_scalar(INH[:, :, 0:9], P[:, :, 0:9],
                                    xsc[:, 1:2], None, A.is_le)
            nc.vector.tensor_scalar(INH[:, :, 9:18], P[:, :, 9:18],
                                    136.0, None, A.is_le)
            nc.vector.tensor_tensor(INR, INR, INH, A.mult)
            FRV = cpool.tile([128, 16, 18], f32, tag="FRV")
            nc.vector.tensor_tensor(FRV, FR, INR, A.mult)
            QC = cpool.tile([128, 16, 18], f32, tag="QC")
            nc.vector.tensor_scalar(QC[:, :, 0:9], Qf[:, :, 0:9],
                                    4.0, 76.0, A.max, A.min)
            nc.vector.tensor_scalar(QC[:, :, 9:18], Qf[:, :, 9:18],
                                    8.0, 137.0, A.max, A.min)
            LINF = cpool.tile([128, 16, 9], f32, tag="LINF")
            nc.vector.tensor_scalar(LINF, QC[:, :, 0:9], 132.0, -536.0,
                                    A.mult, A.add)
            nc.vector.tensor_tensor(LINF, LINF, QC[:, :, 9:18], A.add)
            # n-major int16 idx [128 w, 9 n, 16 t]
            gpre = ipool.tile([128, 9, 16], i16, tag="gpre")
            nc.vector.tensor_copy(
                gpre,
                bass.AP(tensor=LINF.tensor, offset=LINF.offset,
                        ap=[LINF.ap[0], [1, 9], [9, 16]]))

            # ---------- idx relayout to wrapped-16 via DRAM ----------
            gst_off = tcn * 1152
            for ph in range(8):
                sl = gpre[ph * 16:ph * 16 + 16]
                nc.sync.dma_start(
                    out=bass.AP(tensor=gstage, offset=gst_off + ph * 144,
                                ap=[[4608, 16], [1, 144]]),
                    in_=bass.AP(tensor=sl.tensor, offset=sl.offset,
                                ap=[sl.ap[0], [1, 144]]))
            sg = ipool.tile([128, 8, 144], i16, tag="sg")
            for grp in range(8):
                nc.sync.dma_start(
                    out=sg[grp * 16:(grp + 1) * 16, :, :],
                    in_=bass.AP(tensor=gstage, offset=gst_off,
                                ap=[[4608, 16], [144, 8], [1, 144]]))
            # gidx[p, k, (ng, tt, ph)] = sg[p, ph, (3k+ng)*16+tt]
            gidx = ipool.tile([128, 3, 384], i16, tag="gidx")
            for k in range(3):
                nc.vector.tensor_copy(
                    bass.AP(tensor=gidx.tensor, offset=gidx.offset + k * 384,
                            ap=[gidx.ap[0], [128, 3], [8, 16], [1, 8]]),
                    bass.AP(tensor=sg.tensor, offset=sg.offset + k * 3 * 16,
                            ap=[sg.ap[0], [16, 3], [1, 16], [144, 8]]))

            # ---------- phase D: gather + combine + final conv ----------
            jA = jpool.tile([16, 8], i16, tag="jA")
            nc.gpsimd.tensor_copy(jA[0:16, 0:4], gidx[0:16, 0, 0:4])
            gs = []
            for k in range(3):
                g = gpool.tile([128, 3, 16, 2, 2, 64], bf16, tag="g")
                nc.gpsimd.dma_gather(
                    out_ap=g.rearrange("p a b c d e -> p (a b) (c d e)"),
                    in_ap=xd2[:, :],
                    idxs_ap=gidx[:, k, :],
                    num_idxs=6144,
                    num_idxs_reg=6144,
                    elem_size=256,
                    single_packet=False,
                    queue_num=k % 4,
                )
                gs.append(g)

            xoff = xopool.tile([128, 16, 9, 64], bf16, tag="xoff")
            for k in range(3):
                g = gs[k]
                for ng in range(3):
                    n = 3 * k + ng
                    D = dpool.tile([128, 16, 2, 64], bf16, tag="D")
                    nc.vector.tensor_tensor(
                        D, g[:, ng, :, 1, :, :], g[:, ng, :, 0, :, :],
                        A.subtract)
                    H = dpool.tile([128, 16, 2, 64], bf16, tag="H")
                    for tt in range(16):
                        nc.vector.scalar_tensor_tensor(
                            H[:, tt, :, :], D[:, tt, :, :],
                            FRV[:, tt, n:n + 1], g[:, ng, tt, 0, :, :],
                            A.mult, A.add)
                    D2 = dpool.tile([128, 16, 64], bf16, tag="D2")
                    nc.vector.tensor_tensor(
                        D2, H[:, :, 1, :], H[:, :, 0, :], A.subtract)
                    for tt in range(16):
                        nc.vector.scalar_tensor_tensor(
                            xoff[:, tt, n, :], D2[:, tt, :],
                            FRV[:, tt, 9 + n:10 + n], H[:, tt, 0, :],
                            A.mult, A.add)

            # final conv: groups of 4 rows, 512-wide matmuls
            outb = opool.tile([64, 4, 512], f32, tag="outb")
            for g4 in range(4):
                rhs = rpool.tile([128, 5, 512], bf16, tag="rhs")
                for jc in range(4):
                    psx = ps_x.tile([128, 4, 128], bf16, tag="x")
                    for ti in range(4):
                        tt = g4 * 4 + ti
                        nc.tensor.transpose(
                            psx[:, ti, :],
                            xoff[:, tt, 2 * jc:2 * jc + 2, :].rearrange(
                                "p a b -> p (a b)"),
                            idb)
                    nc.any.tensor_copy(
                        rhs[:, jc, :].rearrange("p (a b) -> p a b", a=4), psx)
                psx9 = ps_x.tile([128, 4, 128], bf16, tag="x")
                for ti in range(4):
                    tt = g4 * 4 + ti
                    nc.tensor.transpose(
                        psx9[0:64, ti, :], xoff[:, tt, 8, :], idb)
                nc.any.tensor_copy(
                    rhs[0:64, 4, :].rearrange("p (a b) -> p a b", a=4),
                    psx9[0:64, :, :])
                pso = ps_o.tile([64, 512], f32, tag="o")
                for jc in range(4):
                    nc.tensor.matmul(pso, wca[:, jc * 64:(jc + 1) * 64],
                                     rhs[:, jc, :], start=(jc == 0), stop=False)
                nc.tensor.matmul(pso, wcb, rhs[0:64, 4, :],
                                 start=False, stop=True)
                nc.any.tensor_copy(outb[:, g4, :], pso)

            nc.sync.dma_start(
                out=out_p[:, tcn * 2048:(tcn + 1) * 2048],
                in_=outb.rearrange("c a b -> c (a b)"))

    nc.finalize()
    _PROGRAM = nc
    return nc


def _host_consts(W_off, b_off, W_conv):
    idxr = np.concatenate([np.arange(0, 18, 2), np.arange(1, 18, 2)])
    W_off_r = W_off[idxr]            # (18, 64, 3, 3)
    b_off_r = b_off[idxr]            # (18,)
    woff = np.ascontiguousarray(
        W_off_r.transpose(2, 3, 1, 0).reshape(9, 64, 18).transpose(1, 0, 2)
    ).reshape(64, 9 * 18).astype(BF16)
    # base2 [128 w, 64 t, 18]
    nidx = np.arange(9)
    pnx = (nidx // 3) - 1
    pny = (nidx % 3) - 1
    tt = np.arange(64)
    ww = np.arange(128)
    base2 = np.zeros((128, 64, 18), np.float32)
    base2[:, :, 0:9] = tt[None, :, None] + 9 + pnx[None, None, :] + \
        b_off_r[None, None, 0:9]
    base2[:, :, 9:18] = ww[:, None, None] + 9 + pny[None, None, :] + \
        b_off_r[None, None, 9:18]
    base2 = base2.reshape(128, 64 * 18)
    # final conv weights
    Wmat = W_conv.reshape(64, 64, 9).transpose(0, 2, 1)   # (co, n, ci)
    wca = np.zeros((128, 256), np.float32)
    for jc in range(4):
        for dn in range(2):
            wca[dn * 64:(dn + 1) * 64, jc * 64:(jc + 1) * 64] = \
                Wmat[:, 2 * jc + dn, :].T
    wcb = np.ascontiguousarray(Wmat[:, 8, :].T)           # (ci, co)
    return {
        "woff": woff,
        "base2": base2,
        "wconv_a": wca.astype(BF16),
        "wconv_b": wcb.astype(BF16),
        "ident_f": np.eye(128, dtype=np.float32),
        "ident_b": np.eye(128, dtype=np.float32).astype(BF16),
    }


def _per_core_inputs(x, consts, s, half):
    h0 = 64 * half
    xs = x[s]                                    # (64, 128, 128)
    xgs = np.zeros((64, NR, 128), BF16)
    lo = h0 - 5                                  # unpadded row of slab row 0
    r0 = max(0, lo)
    r1 = min(128, lo + NR)
    xgs[:, r0 - lo:r1 - lo, :] = xs[:, r0:r1, :].astype(BF16)
    xsc = np.zeros((128, 2), np.float32)
    xsc[:, 0] = 9 - h0                           # x mask lo
    xsc[:, 1] = 136 - h0                         # x mask hi
    return {
        "xg": xgs.reshape(64, NR * 128),
        "xsc": xsc,
        **consts,
    }


def kernel(x, W_off, b_off, W_conv):
    _install_ntff_hook()
    # the bass kernel must run on the axon trn2 backend; undo any cpu pin
    import os
    if os.environ.get("JAX_PLATFORMS", "") == "cpu":
        try:
            import jax
            jax.config.update("jax_platforms", None)
            os.environ.pop("JAX_PLATFORMS", None)
        except Exception:
            pass
    x = np.asarray(x, np.float32)
    W_off = np.asarray(W_off, np.float32)
    b_off = np.asarray(b_off, np.float32)
    W_conv = np.asarray(W_conv, np.float32)

    from concourse.bass_utils import run_bass_kernel_spmd
    nc = _build_program()
    consts = _host_consts(W_off, b_off, W_conv)
    in_maps = [
        _per_core_inputs(x, consts, core // 2, core % 2) for core in range(NCORES)
    ]
    res = run_bass_kernel_spmd(nc, in_maps, list(range(NCORES)))
    out = np.empty((4, 64, 128, 128), np.float32)
    for core in range(NCORES):
        s, half = core // 2, core % 2
        out[s, :, 64 * half:64 * half + 64, :] = \
            res.results[core]["out"].reshape(64, 64, 128)
    return out
